# revision 1
# baseline (speedup 1.0000x reference)
"""RetinaFace-style multi-task loss on 8 Trainium NeuronCores.

Data-parallel: batch dim (16) sharded 2 samples/core across 8 cores via pmap;
anchors replicated. XLA-friendly formulation:
  - hard-negative top-k sum via threshold bisection (no sort)
  - argmax via arithmetic first-min-index (no argmax lowering issues)
  - GT gather via one-hot matmul (no dynamic gather)
All loss math follows the reference bit-closely in fp32.
"""
import numpy as np

_B, _A, _N = 16, 102400, 32
_NC = 8
_OMEGA, _EPS = 3.0, 2.0
_WING_C = _OMEGA - _OMEGA * float(np.log(1.0 + _OMEGA / _EPS))

_pfn = None


def _build():
    global _pfn
    if _pfn is not None:
        return _pfn
    import jax
    import jax.numpy as jnp

    def per_sample(cls, breg, lreg, ann, anchor):
        aw = anchor[:, 2] - anchor[:, 0]
        ah = anchor[:, 3] - anchor[:, 1]
        acx = anchor[:, 0] + 0.5 * aw
        acy = anchor[:, 1] + 0.5 * ah
        valid = ann[:, 0] > 0
        boxes = ann[:, :4]
        ldm_ann = ann[:, 4:]
        has_gt = jnp.any(valid)

        # IoU [A, 32], invalid GT masked to -1 (as in reference)
        barea = (boxes[:, 2] - boxes[:, 0]) * (boxes[:, 3] - boxes[:, 1])
        iw = jnp.minimum(anchor[:, 2][:, None], boxes[None, :, 2]) - jnp.maximum(
            anchor[:, 0][:, None], boxes[None, :, 0])
        ih = jnp.minimum(anchor[:, 3][:, None], boxes[None, :, 3]) - jnp.maximum(
            anchor[:, 1][:, None], boxes[None, :, 1])
        iw = jnp.clip(iw, 0.0)
        ih = jnp.clip(ih, 0.0)
        ua = (aw * ah)[:, None] + barea[None, :] - iw * ih
        ua = jnp.clip(ua, 1e-8)
        iou = iw * ih / ua
        iou = jnp.where(valid[None, :], iou, -1.0)
        iou_max = iou.max(axis=1)

        # first-occurrence argmax, arithmetically (min index attaining max)
        j32 = jnp.arange(32, dtype=jnp.int32)
        idxs = jnp.where(iou == iou_max[:, None], j32[None, :], 99)
        iou_arg = idxs.min(axis=1)
        onehot = (j32[None, :] == iou_arg[:, None]).astype(jnp.float32)

        neg = iou_max < 0.4
        pos = iou_max >= 0.7
        npos = pos.sum()
        nneg = neg.sum()
        keep = jnp.minimum(nneg, 3 * npos)

        # hard-negative mining: sum of top-`keep` of neg scores via bisection
        v = jnp.where(neg, -cls[:, 1], jnp.float32(-1e4))

        def body(_, s):
            lo, hi = s
            mid = 0.5 * (lo + hi)
            big = (v >= mid).sum() >= keep
            return (jnp.where(big, mid, lo), jnp.where(big, hi, mid))

        lo, hi = jax.lax.fori_loop(
            0, 48, body, (jnp.float32(-1e4), jnp.float32(64.0)))
        c_lo = ((v >= lo).sum()).astype(jnp.float32)
        s_lo = jnp.where(v >= lo, v, 0.0).sum()
        keep_f = keep.astype(jnp.float32)
        neg_sum = s_lo - (c_lo - keep_f) * lo
        neg_mean = neg_sum / jnp.maximum(keep_f, 1.0)
        pos_mean = jnp.where(pos, -cls[:, 0], 0.0).sum() / jnp.maximum(npos, 1)
        cls_loss = jnp.where(has_gt & (npos > 0), pos_mean + neg_mean, 0.0)

        # bbox: SmoothL1 on positives
        gb = jnp.einsum('aj,jk->ak', onehot, boxes,
                        preferred_element_type=jnp.float32)
        gw = gb[:, 2] - gb[:, 0]
        gh = gb[:, 3] - gb[:, 1]
        gcx = gb[:, 0] + 0.5 * gw
        gcy = gb[:, 1] + 0.5 * gh
        tdx = (gcx - acx) / (aw + 1e-14)
        tdy = (gcy - acy) / (ah + 1e-14)
        safe_rw = jnp.where(gw > 0, gw / aw, 1.0)
        safe_rh = jnp.where(gh > 0, gh / ah, 1.0)
        tdw = jnp.log(safe_rw)
        tdh = jnp.log(safe_rh)
        bbox_scale = jnp.array([0.1, 0.1, 0.2, 0.2], jnp.float32)
        btgt = jnp.stack([tdx, tdy, tdw, tdh], axis=1) / bbox_scale
        d = jnp.abs(btgt - breg)
        sl1 = jnp.where(d < 1.0, 0.5 * d * d, d - 0.5)
        bbox_loss = jnp.where(
            has_gt & (npos > 0),
            jnp.where(pos[:, None], sl1, 0.0).sum() / (jnp.maximum(npos, 1) * 4),
            0.0)

        # landmarks: wing loss on positives whose GT has landmarks
        even = (jnp.arange(196) % 2) == 0
        ctr = jnp.where(even, acx[:, None], acy[:, None])
        den = jnp.where(even, aw[:, None], ah[:, None]) + 1e-14
        s = jnp.concatenate(
            [jnp.ones(68, jnp.float32), 3.0 * jnp.ones(128, jnp.float32)])
        gl = jnp.einsum('aj,jk->ak', onehot, ldm_ann,
                        preferred_element_type=jnp.float32)
        lpos = pos & (gl.sum(axis=1) > 0)
        nl = lpos.sum()
        ltgt = (gl - ctr) / den / 0.1
        dd = jnp.abs(ltgt * s - lreg * s)
        wing = jnp.where(dd < _OMEGA, _OMEGA * jnp.log1p(dd / _EPS), dd - _WING_C)
        ldm_loss = jnp.where(
            has_gt & (nl > 0),
            jnp.where(lpos[:, None], wing, 0.0).sum() / (jnp.maximum(nl, 1) * 196),
            0.0)
        return cls_loss, bbox_loss, ldm_loss

    def per_core(cls, breg, lreg, ann, anchor):
        return jax.vmap(per_sample, in_axes=(0, 0, 0, 0, None))(
            cls, breg, lreg, ann, anchor)

    _pfn = jax.pmap(per_core, in_axes=(0, 0, 0, 0, 0))
    return _pfn


def kernel(classifications, bbox_regressions, ldm_regressions, anchors,
           annotations):
    fn = _build()
    spb = _B // _NC  # samples per core
    cls = np.asarray(classifications, np.float32).reshape(_NC, spb, _A, 2)
    breg = np.asarray(bbox_regressions, np.float32).reshape(_NC, spb, _A, 4)
    lreg = np.asarray(ldm_regressions, np.float32).reshape(_NC, spb, _A, 196)
    ann = np.asarray(annotations, np.float32).reshape(_NC, spb, _N, 200)
    anc = np.broadcast_to(np.asarray(anchors, np.float32)[0], (_NC, _A, 4))
    cl, bb, ld = fn(cls, breg, lreg, ann, anc)
    return (np.asarray(cl, np.float32).reshape(_B),
            np.asarray(bb, np.float32).reshape(_B),
            np.asarray(ld, np.float32).reshape(_B))


# revision 2
# speedup vs baseline: 58.5688x; 58.5688x over previous
"""RetinaFace-style multi-task loss on 8 Trainium NeuronCores (data-parallel).

Two-phase device pipeline to avoid shipping the 1.25 GB ldm_regressions tensor
through the axon tunnel when only ~200 positive-anchor rows per sample
contribute to the bbox/landmark losses:

  Phase A (device, pmap over 8 cores x 2 samples): full A x 32 IoU,
    pos/neg classification, hard-negative-mined classification loss
    (top-k sum via threshold bisection -- no sort), exports pos mask +
    matched-GT index per anchor (uint8).
  Host: compact positive indices, gather the needed rows of
    bbox_regressions / ldm_regressions / anchors / GT boxes / GT landmarks.
  Phase B (device, pmap): SmoothL1 bbox loss + wing landmark loss on the
    gathered [256]-row tiles.

All math fp32, mirroring the reference formulas exactly.
"""
import numpy as np

_B, _A, _N = 16, 102400, 32
_NC = 8
_K = 256  # max positives per sample (observed ~200; asserted at runtime)
_OMEGA, _EPS = 3.0, 2.0
_WING_C = _OMEGA - _OMEGA * float(np.log(1.0 + _OMEGA / _EPS))

_fns = None


def _build():
    global _fns
    if _fns is not None:
        return _fns
    import jax
    import jax.numpy as jnp

    # ---------------- phase A ----------------
    def phase_a(cls, ann, anchor):
        # cls [A,2], ann [32,200], anchor [A,4]
        aw = anchor[:, 2] - anchor[:, 0]
        ah = anchor[:, 3] - anchor[:, 1]
        valid = ann[:, 0] > 0
        boxes = ann[:, :4]
        has_gt = jnp.any(valid)

        barea = (boxes[:, 2] - boxes[:, 0]) * (boxes[:, 3] - boxes[:, 1])
        iw = jnp.minimum(anchor[:, 2][:, None], boxes[None, :, 2]) - jnp.maximum(
            anchor[:, 0][:, None], boxes[None, :, 0])
        ih = jnp.minimum(anchor[:, 3][:, None], boxes[None, :, 3]) - jnp.maximum(
            anchor[:, 1][:, None], boxes[None, :, 1])
        iw = jnp.clip(iw, 0.0)
        ih = jnp.clip(ih, 0.0)
        ua = jnp.clip((aw * ah)[:, None] + barea[None, :] - iw * ih, 1e-8)
        iou = iw * ih / ua
        iou = jnp.where(valid[None, :], iou, -1.0)
        iou_max = iou.max(axis=1)

        j32 = jnp.arange(32, dtype=jnp.int32)
        idxs = jnp.where(iou == iou_max[:, None], j32[None, :], 99)
        iou_arg = idxs.min(axis=1).astype(jnp.uint8)

        neg = iou_max < 0.4
        pos = iou_max >= 0.7
        npos = pos.sum()
        nneg = neg.sum()
        keep = jnp.minimum(nneg, 3 * npos)

        v = jnp.where(neg, -cls[:, 1], jnp.float32(-1e4))

        def body(_, s):
            lo, hi = s
            mid = 0.5 * (lo + hi)
            big = (v >= mid).sum() >= keep
            return (jnp.where(big, mid, lo), jnp.where(big, hi, mid))

        lo, _hi = jax.lax.fori_loop(
            0, 48, body, (jnp.float32(-1e4), jnp.float32(64.0)))
        c_lo = ((v >= lo).sum()).astype(jnp.float32)
        s_lo = jnp.where(v >= lo, v, 0.0).sum()
        keep_f = keep.astype(jnp.float32)
        neg_mean = (s_lo - (c_lo - keep_f) * lo) / jnp.maximum(keep_f, 1.0)
        pos_mean = jnp.where(pos, -cls[:, 0], 0.0).sum() / jnp.maximum(npos, 1)
        cls_loss = jnp.where(has_gt & (npos > 0), pos_mean + neg_mean, 0.0)
        return cls_loss, pos.astype(jnp.uint8), iou_arg

    def phase_a_core(cls, ann, anchor):
        return jax.vmap(phase_a, in_axes=(0, 0, None))(cls, ann, anchor)

    # ---------------- phase B ----------------
    def phase_b(breg, lreg, anc, gb, gl, rowv, npos, has_gt):
        # [K,4] [K,196] [K,4] [K,4] [K,196] [K] scalars
        aw = anc[:, 2] - anc[:, 0]
        ah = anc[:, 3] - anc[:, 1]
        acx = anc[:, 0] + 0.5 * aw
        acy = anc[:, 1] + 0.5 * ah
        gw = gb[:, 2] - gb[:, 0]
        gh = gb[:, 3] - gb[:, 1]
        gcx = gb[:, 0] + 0.5 * gw
        gcy = gb[:, 1] + 0.5 * gh
        tdx = (gcx - acx) / (aw + 1e-14)
        tdy = (gcy - acy) / (ah + 1e-14)
        tdw = jnp.log(jnp.where(gw > 0, gw / aw, 1.0))
        tdh = jnp.log(jnp.where(gh > 0, gh / ah, 1.0))
        bbox_scale = jnp.array([0.1, 0.1, 0.2, 0.2], jnp.float32)
        btgt = jnp.stack([tdx, tdy, tdw, tdh], axis=1) / bbox_scale
        d = jnp.abs(btgt - breg)
        sl1 = jnp.where(d < 1.0, 0.5 * d * d, d - 0.5)
        npos_f = jnp.maximum(npos, 1.0)
        bbox_loss = jnp.where(
            (has_gt > 0) & (npos > 0),
            jnp.where(rowv[:, None] > 0, sl1, 0.0).sum() / (npos_f * 4.0), 0.0)

        even = (jnp.arange(196) % 2) == 0
        ctr = jnp.where(even, acx[:, None], acy[:, None])
        den = jnp.where(even, aw[:, None], ah[:, None]) + 1e-14
        s = jnp.concatenate(
            [jnp.ones(68, jnp.float32), 3.0 * jnp.ones(128, jnp.float32)])
        lposv = (rowv > 0) & (gl.sum(axis=1) > 0)
        nl = lposv.sum()
        ltgt = (gl - ctr) / den / 0.1
        dd = jnp.abs(ltgt * s - lreg * s)
        wing = jnp.where(dd < _OMEGA, _OMEGA * jnp.log1p(dd / _EPS), dd - _WING_C)
        ldm_loss = jnp.where(
            (has_gt > 0) & (nl > 0),
            jnp.where(lposv[:, None], wing, 0.0).sum() /
            (jnp.maximum(nl, 1) * 196), 0.0)
        return bbox_loss, ldm_loss

    def phase_b_core(*a):
        return jax.vmap(phase_b)(*a)

    _fns = (jax.pmap(phase_a_core, in_axes=(0, 0, 0)),
            jax.pmap(phase_b_core))
    return _fns


def kernel(classifications, bbox_regressions, ldm_regressions, anchors,
           annotations):
    fa, fb = _build()
    spb = _B // _NC
    cls = np.asarray(classifications, np.float32).reshape(_NC, spb, _A, 2)
    ann_h = np.asarray(annotations, np.float32)
    ann = ann_h.reshape(_NC, spb, _N, 200)
    anc_full = np.asarray(anchors, np.float32)[0]
    anc8 = np.broadcast_to(anc_full, (_NC, _A, 4))

    cls_loss, pos_u8, arg_u8 = fa(cls, ann, anc8)
    cls_loss = np.asarray(cls_loss, np.float32).reshape(_B)
    pos_m = np.asarray(pos_u8).reshape(_B, _A)
    arg_m = np.asarray(arg_u8).reshape(_B, _A)

    breg_h = np.asarray(bbox_regressions, np.float32)
    lreg_h = np.asarray(ldm_regressions, np.float32)

    breg_g = np.zeros((_B, _K, 4), np.float32)
    lreg_g = np.zeros((_B, _K, 196), np.float32)
    anc_g = np.zeros((_B, _K, 4), np.float32)
    anc_g[:, :, 2:] = 1.0  # pad anchors with w=h=1 to keep logs finite
    gb_g = np.zeros((_B, _K, 4), np.float32)
    gb_g[:, :, 2:] = 1.0
    gl_g = np.zeros((_B, _K, 196), np.float32)
    rowv = np.zeros((_B, _K), np.float32)
    nposs = np.zeros((_B,), np.float32)
    hasgt = np.zeros((_B,), np.float32)

    for b in range(_B):
        idx = np.nonzero(pos_m[b])[0]
        n = idx.size
        assert n <= _K, f'npos={n} exceeds K={_K}'
        gt = arg_m[b, idx].astype(np.int64)
        breg_g[b, :n] = breg_h[b, idx]
        lreg_g[b, :n] = lreg_h[b, idx]
        anc_g[b, :n] = anc_full[idx]
        gb_g[b, :n] = ann_h[b, gt, :4]
        gl_g[b, :n] = ann_h[b, gt, 4:]
        rowv[b, :n] = 1.0
        nposs[b] = n
        hasgt[b] = float((ann_h[b, :, 0] > 0).any())

    sh = lambda x: x.reshape((_NC, spb) + x.shape[1:])
    bbox_loss, ldm_loss = fb(sh(breg_g), sh(lreg_g), sh(anc_g), sh(gb_g),
                             sh(gl_g), sh(rowv), sh(nposs), sh(hasgt))
    return (cls_loss,
            np.asarray(bbox_loss, np.float32).reshape(_B),
            np.asarray(ldm_loss, np.float32).reshape(_B))


# revision 3
# speedup vs baseline: 66.3585x; 1.1330x over previous
"""RetinaFace-style multi-task loss on 8 Trainium NeuronCores (data-parallel).

Two-phase device pipeline to avoid shipping the 1.25 GB ldm_regressions tensor
through the interconnect when only ~200 positive-anchor rows per sample
contribute to the bbox/landmark losses:

  Phase A (device, pmap over 8 cores x 2 samples): full A x 32 IoU,
    pos/neg classification, hard-negative-mined classification loss
    (top-k sum via 16-way threshold search -- no sort). Exports a single
    uint8 plane per anchor: matched-GT index (low bits) | pos flag (bit 7).
  Host: compact positive indices, slice the needed rows of
    bbox_regressions / ldm_regressions / anchors.
  Phase B (device, pmap): gathers GT boxes/landmarks from annotations via
    one-hot matmul, SmoothL1 bbox loss + wing landmark loss on [256]-row tiles.

All math fp32, mirroring the reference formulas.
"""
import numpy as np

_B, _A, _N = 16, 102400, 32
_NC = 8
_K = 256  # max positives per sample (observed ~200; asserted at runtime)
_OMEGA, _EPS = 3.0, 2.0
_WING_C = _OMEGA - _OMEGA * float(np.log(1.0 + _OMEGA / _EPS))

_fns = None


def _build():
    global _fns
    if _fns is not None:
        return _fns
    import jax
    import jax.numpy as jnp

    # ---------------- phase A ----------------
    def phase_a(cls, ann, anchor):
        # cls [A,2], ann [32,200], anchor [A,4]
        aw = anchor[:, 2] - anchor[:, 0]
        ah = anchor[:, 3] - anchor[:, 1]
        valid = ann[:, 0] > 0
        boxes = ann[:, :4]
        has_gt = jnp.any(valid)

        barea = (boxes[:, 2] - boxes[:, 0]) * (boxes[:, 3] - boxes[:, 1])
        iw = jnp.minimum(anchor[:, 2][:, None], boxes[None, :, 2]) - jnp.maximum(
            anchor[:, 0][:, None], boxes[None, :, 0])
        ih = jnp.minimum(anchor[:, 3][:, None], boxes[None, :, 3]) - jnp.maximum(
            anchor[:, 1][:, None], boxes[None, :, 1])
        iw = jnp.clip(iw, 0.0)
        ih = jnp.clip(ih, 0.0)
        ua = jnp.clip((aw * ah)[:, None] + barea[None, :] - iw * ih, 1e-8)
        iou = iw * ih / ua
        iou = jnp.where(valid[None, :], iou, -1.0)
        iou_max = iou.max(axis=1)

        j32 = jnp.arange(32, dtype=jnp.int32)
        idxs = jnp.where(iou == iou_max[:, None], j32[None, :], 99)
        iou_arg = idxs.min(axis=1)

        neg = iou_max < 0.4
        pos = iou_max >= 0.7
        packed = (iou_arg.astype(jnp.uint8)
                  | (pos.astype(jnp.uint8) << 7))
        npos = pos.sum()
        nneg = neg.sum()
        keep = jnp.minimum(nneg, 3 * npos)

        # hard-negative mining: sum of top-`keep` scores via 16-way search
        v = jnp.where(neg, -cls[:, 1], jnp.float32(-1e2))
        ks = jnp.arange(16, dtype=jnp.float32)

        def body(_, s):
            lo, hi = s
            t = lo + (ks + 1.0) * ((hi - lo) / 17.0)
            c = (v[:, None] >= t[None, :]).sum(axis=0)
            big = c >= keep
            lo2 = jnp.max(jnp.where(big, t, lo))
            hi2 = jnp.min(jnp.where(big, hi, t))
            return lo2, hi2

        lo, _hi = jax.lax.fori_loop(
            0, 5, body, (jnp.float32(-1e2), jnp.float32(64.0)))
        c_lo = ((v >= lo).sum()).astype(jnp.float32)
        s_lo = jnp.where(v >= lo, v, 0.0).sum()
        keep_f = keep.astype(jnp.float32)
        neg_mean = (s_lo - (c_lo - keep_f) * lo) / jnp.maximum(keep_f, 1.0)
        pos_mean = jnp.where(pos, -cls[:, 0], 0.0).sum() / jnp.maximum(npos, 1)
        cls_loss = jnp.where(has_gt & (npos > 0), pos_mean + neg_mean, 0.0)
        return cls_loss, packed

    def phase_a_core(cls, ann, anchor):
        return jax.vmap(phase_a, in_axes=(0, 0, None))(cls, ann, anchor)

    # ---------------- phase B ----------------
    def phase_b(breg, lreg, anc, ann, gt, rowv, npos, has_gt):
        # breg [K,4], lreg [K,196], anc [K,4], ann [32,200], gt [K] int32
        onehot = (jnp.arange(32, dtype=jnp.int32)[None, :]
                  == gt[:, None]).astype(jnp.float32)
        gb = jnp.einsum('kj,jc->kc', onehot, ann[:, :4],
                        preferred_element_type=jnp.float32)
        gl = jnp.einsum('kj,jc->kc', onehot, ann[:, 4:],
                        preferred_element_type=jnp.float32)

        aw = anc[:, 2] - anc[:, 0]
        ah = anc[:, 3] - anc[:, 1]
        acx = anc[:, 0] + 0.5 * aw
        acy = anc[:, 1] + 0.5 * ah
        gw = gb[:, 2] - gb[:, 0]
        gh = gb[:, 3] - gb[:, 1]
        gcx = gb[:, 0] + 0.5 * gw
        gcy = gb[:, 1] + 0.5 * gh
        tdx = (gcx - acx) / (aw + 1e-14)
        tdy = (gcy - acy) / (ah + 1e-14)
        tdw = jnp.log(jnp.where(gw > 0, gw / aw, 1.0))
        tdh = jnp.log(jnp.where(gh > 0, gh / ah, 1.0))
        bbox_scale = jnp.array([0.1, 0.1, 0.2, 0.2], jnp.float32)
        btgt = jnp.stack([tdx, tdy, tdw, tdh], axis=1) / bbox_scale
        d = jnp.abs(btgt - breg)
        sl1 = jnp.where(d < 1.0, 0.5 * d * d, d - 0.5)
        npos_f = jnp.maximum(npos, 1.0)
        bbox_loss = jnp.where(
            (has_gt > 0) & (npos > 0),
            jnp.where(rowv[:, None] > 0, sl1, 0.0).sum() / (npos_f * 4.0), 0.0)

        even = (jnp.arange(196) % 2) == 0
        ctr = jnp.where(even, acx[:, None], acy[:, None])
        den = jnp.where(even, aw[:, None], ah[:, None]) + 1e-14
        s = jnp.concatenate(
            [jnp.ones(68, jnp.float32), 3.0 * jnp.ones(128, jnp.float32)])
        lposv = (rowv > 0) & (gl.sum(axis=1) > 0)
        nl = lposv.sum()
        ltgt = (gl - ctr) / den / 0.1
        dd = jnp.abs(ltgt * s - lreg * s)
        wing = jnp.where(dd < _OMEGA, _OMEGA * jnp.log1p(dd / _EPS), dd - _WING_C)
        ldm_loss = jnp.where(
            (has_gt > 0) & (nl > 0),
            jnp.where(lposv[:, None], wing, 0.0).sum() /
            (jnp.maximum(nl, 1) * 196), 0.0)
        return bbox_loss, ldm_loss

    def phase_b_core(*a):
        return jax.vmap(phase_b)(*a)

    _fns = (jax.pmap(phase_a_core, in_axes=(0, 0, 0)),
            jax.pmap(phase_b_core))
    return _fns


def kernel(classifications, bbox_regressions, ldm_regressions, anchors,
           annotations):
    fa, fb = _build()
    spb = _B // _NC
    cls = np.asarray(classifications, np.float32).reshape(_NC, spb, _A, 2)
    ann_h = np.asarray(annotations, np.float32)
    ann = ann_h.reshape(_NC, spb, _N, 200)
    anc_full = np.asarray(anchors, np.float32)[0]
    anc8 = np.broadcast_to(anc_full, (_NC, _A, 4))

    cls_loss, packed = fa(cls, ann, anc8)
    cls_loss = np.asarray(cls_loss, np.float32).reshape(_B)
    packed = np.asarray(packed).reshape(_B, _A)
    pos_m = (packed >> 7) & 1
    arg_m = packed & 0x3F

    breg_h = np.asarray(bbox_regressions, np.float32)
    lreg_h = np.asarray(ldm_regressions, np.float32)

    breg_g = np.zeros((_B, _K, 4), np.float32)
    lreg_g = np.zeros((_B, _K, 196), np.float32)
    anc_g = np.zeros((_B, _K, 4), np.float32)
    anc_g[:, :, 2:] = 1.0  # pad anchors keep logs/denominators finite
    gt_g = np.full((_B, _K), 99, np.int32)  # 99 -> all-zero one-hot row
    rowv = np.zeros((_B, _K), np.float32)
    nposs = np.zeros((_B,), np.float32)
    hasgt = np.zeros((_B,), np.float32)

    for b in range(_B):
        idx = np.nonzero(pos_m[b])[0]
        n = idx.size
        assert n <= _K, f'npos={n} exceeds K={_K}'
        breg_g[b, :n] = breg_h[b, idx]
        lreg_g[b, :n] = lreg_h[b, idx]
        anc_g[b, :n] = anc_full[idx]
        gt_g[b, :n] = arg_m[b, idx]
        rowv[b, :n] = 1.0
        nposs[b] = n
        hasgt[b] = float((ann_h[b, :, 0] > 0).any())

    sh = lambda x: x.reshape((_NC, spb) + x.shape[1:])
    bbox_loss, ldm_loss = fb(sh(breg_g), sh(lreg_g), sh(anc_g), sh(ann_h),
                             sh(gt_g), sh(rowv), sh(nposs), sh(hasgt))
    return (cls_loss,
            np.asarray(bbox_loss, np.float32).reshape(_B),
            np.asarray(ldm_loss, np.float32).reshape(_B))
